# revision 1
# baseline (speedup 1.0000x reference)
"""Distributed Trainium2 Bass kernel: RMSNorm + 16-head attention + out-proj.

Problem (hardcoded): tokens [4, 2048, 2048] f32, DIM=2048, HEADS=16, DHEAD=128.
Sharding: 8 cores = 4 batches x 2 sequence halves. Each core owns 1024 query
rows of one batch; K/V for the full 2048 keys arrive via pair-wise AllGathers
(cores 2i and 2i+1 hold the two halves of batch i). All matmuls run in bf16
with fp32 PSUM accumulation; the rmsnorm statistics are computed in fp32 and
folded into a PE transpose (x.T @ diag(rstd)).

Host-side folding: norm_weight and the q-scale (dhead^-0.5) are folded into
the projection weight matrices.

Queue discipline: weight DMAs go on nc.sync / nc.scalar (HWDGE) and complete
before the collectives; the gather-dependent attention loads go on nc.gpsimd
(SWDGE rings) so a wait on the collective semaphore never blocks anything
through shared HWDGE ring flow-control.
"""

import os
import sys

for p in ("/opt/trn_rl_repo", "/root/.axon_site/_ro/trn_rl_repo"):
    if os.path.isdir(p) and p not in sys.path:
        sys.path.append(p)

import numpy as np
import ml_dtypes

BF16 = ml_dtypes.bfloat16

DIM = 2048
HEADS = 16
DHEAD = 128
B = 4
N = 2048
NCORES = 8
LOCAL = N // 2          # 1024 rows per core
P = 128                 # partitions
MC = DIM // P           # 16 model chunks
RT = LOCAL // P         # 8 row tiles
QH = 2                  # qrow halves of 512
FD = 512                # moving free dim
KCHUNKS = N // P        # 16 key chunks (over both blocks)
OC = DIM // FD          # 4 out-col chunks
HH = HEADS // 2         # 8 heads per gather half
EPS = 1.1920929e-07

_CACHED_NC = None


def build():
    from concourse import bacc, tile, mybir
    from concourse.masks import make_identity

    fp32 = mybir.dt.float32
    bf16 = mybir.dt.bfloat16

    nc = bacc.Bacc(
        "TRN2", target_bir_lowering=False, debug=False, num_devices=NCORES
    )

    toks = nc.dram_tensor("tokens", [LOCAL, DIM], fp32, kind="ExternalInput")
    wq = nc.dram_tensor("wq", [HEADS, P, MC, P], bf16, kind="ExternalInput")
    wk = nc.dram_tensor("wk", [HEADS, P, MC, P], bf16, kind="ExternalInput")
    wv = nc.dram_tensor("wv", [DIM, DIM], bf16, kind="ExternalInput")
    wo = nc.dram_tensor("wo", [DIM, DIM], bf16, kind="ExternalInput")
    out = nc.dram_tensor("out", [LOCAL, DIM], fp32, kind="ExternalOutput")

    # internal DRAM scratch; k split in two head-halves so each half can
    # gather as soon as its projection finishes
    kl_h = [nc.dram_tensor(f"kl{g}", [HH, P, LOCAL], bf16) for g in range(2)]
    vl_h = [nc.dram_tensor(f"vl{g}", [RT, P, DIM // 2], bf16)
            for g in range(2)]                                # local v, h-group
    kg_h = [nc.dram_tensor(f"kg{g}", [2, HH, P, LOCAL], bf16) for g in range(2)]
    vg_h = [nc.dram_tensor(f"vg{g}", [2, RT, P, DIM // 2], bf16)
            for g in range(2)]

    RG = [[0, 1], [2, 3], [4, 5], [6, 7]]

    with tile.TileContext(nc) as tc:
      with tc.tile_pool(name="persist", bufs=1) as persist:
        qt_sb = [persist.tile([P, LOCAL], bf16, tag=f"qt{h}", name=f"qt{h}")
                 for h in range(HEADS)]
        avt_sb = [persist.tile([P, LOCAL], bf16, tag=f"avt{h}", name=f"avt{h}")
                  for h in range(HEADS)]
        ones_sb = persist.tile([P, P], bf16, tag="ones")
        nc.vector.memset(ones_sb[:], 1.0)
        eps_sb = persist.tile([P, 1], fp32, tag="eps")
        nc.vector.memset(eps_sb[:], EPS)

        # ------- Phase 1: rmsnorm fused into a PE transpose (x.T @ diag) ----
        with tc.tile_pool(name="xt", bufs=1) as xtp:
            xT = [xtp.tile([P, LOCAL], bf16, tag=f"xt{m}", name=f"xt{m}")
                  for m in range(MC)]
            with (
                tc.tile_pool(name="p1x", bufs=1) as p1x,
                tc.tile_pool(name="p1", bufs=2) as p1,
                tc.tile_pool(name="stat", bufs=4) as stat,
                tc.tile_pool(name="psT", bufs=4, space="PSUM") as psT,
            ):
                # prefetch all 8 x row-tiles up front on four queues
                xall = p1x.tile([P, RT, DIM], fp32, tag="xall")
                qs = (nc.sync, nc.scalar, nc.gpsimd)
                for rt in range(RT):
                    qs[rt % 3].dma_start(
                        out=xall[:, rt, :], in_=toks[rt * P:(rt + 1) * P, :]
                    )
                ident = p1.tile([P, P], bf16, tag="ident", bufs=1)
                make_identity(nc, ident[:])
                for rt in range(RT):
                    x = xall[:, rt, :]
                    sq = p1.tile([P, DIM], fp32, tag="sq")
                    ssq = stat.tile([P, 1], fp32, tag="ssq")
                    nc.scalar.activation(
                        sq[:], x, mybir.ActivationFunctionType.Square,
                        accum_out=ssq[:],
                    )
                    std = stat.tile([P, 1], fp32, tag="std")
                    nc.scalar.activation(
                        std[:], ssq[:], mybir.ActivationFunctionType.Sqrt,
                        bias=eps_sb[:], scale=1.0 / DIM,
                    )
                    rstd = stat.tile([P, 1], fp32, tag="rstd")
                    nc.vector.reciprocal(rstd[:], std[:])
                    diag = stat.tile([P, P], bf16, tag="diag")
                    nc.vector.tensor_scalar_mul(diag[:], ident[:], rstd[:])
                    xb = p1.tile([P, DIM], bf16, tag="xb")
                    nc.scalar.activation(
                        xb[:], x, mybir.ActivationFunctionType.Copy
                    )
                    # xT[m][:, rows] = (x[:, m-chunk]).T @ diag(rstd)
                    for m in range(MC):
                        pt = psT.tile([P, P], fp32, tag="pt")
                        nc.tensor.matmul(
                            pt[:], xb[:, m * P:(m + 1) * P], diag[:],
                            start=True, stop=True,
                        )
                        nc.vector.tensor_copy(
                            xT[m][:, rt * P:(rt + 1) * P], pt[:]
                        )

            # ---------------- Phase 2: k^T and v projections ---------------
            with (
                tc.tile_pool(name="p2", bufs=2) as p2,
                tc.tile_pool(name="psA", bufs=3, space="PSUM") as psA,
            ):
                wv_full = p2.tile([P, MC, DIM], bf16, tag="wvf", bufs=1)
                nc.scalar.dma_start(
                    out=wv_full[:],
                    in_=wv.ap().rearrange("(mc p) d -> p mc d", p=P),
                )
                # k^T: stationary = wk chunks, moving = xT; each head-half
                # gathers as soon as it is projected
                for g in range(2):
                    for hh in range(HH):
                        h = g * HH + hh
                        wk_sb = p2.tile([P, MC, P], bf16, tag="wk")
                        nc.sync.dma_start(out=wk_sb[:], in_=wk[h])
                        for q in range(QH):
                            ps = psA.tile([P, FD], fp32, tag="pp", bufs=4)
                            for m in range(MC):
                                nc.tensor.matmul(
                                    ps[:], wk_sb[:, m, :],
                                    xT[m][:, q * FD:(q + 1) * FD],
                                    start=(m == 0), stop=(m == MC - 1),
                                )
                            stg = p2.tile([P, FD], bf16, tag="stg", bufs=8)
                            nc.vector.tensor_copy(stg[:], ps[:])
                            nc.sync.dma_start(
                                out=kl_h[g].ap()[hh][:, q * FD:(q + 1) * FD],
                                in_=stg[:],
                            )
                    nc.gpsimd.collective_compute(
                        "AllGather", mybir.AluOpType.bypass,
                        replica_groups=RG,
                        ins=[kl_h[g].ap().opt()],
                        outs=[kg_h[g].ap().opt()],
                    )
                # v: stationary = xT chunks, moving = wv_full[mc, oc] slices
                # gathered per head-group so attention h<8 starts sooner
                for g in range(2):
                    for oc in (2 * g, 2 * g + 1):
                        for rt in range(RT):
                            ps = psA.tile([P, FD], fp32, tag="pp", bufs=4)
                            for m in range(MC):
                                nc.tensor.matmul(
                                    ps[:], xT[m][:, rt * P:(rt + 1) * P],
                                    wv_full[:, m, oc * FD:(oc + 1) * FD],
                                    start=(m == 0), stop=(m == MC - 1),
                                )
                            stg = p2.tile([P, FD], bf16, tag="stg", bufs=8)
                            nc.vector.tensor_copy(stg[:], ps[:])
                            nc.sync.dma_start(
                                out=vl_h[g].ap()[rt][
                                    :, (oc % 2) * FD:(oc % 2 + 1) * FD
                                ],
                                in_=stg[:],
                            )
                    nc.gpsimd.collective_compute(
                        "AllGather", mybir.AluOpType.bypass,
                        replica_groups=RG,
                        ins=[vl_h[g].ap().opt()],
                        outs=[vg_h[g].ap().opt()],
                    )

                # ---------------- Phase 3: q^T projection ------------------
                for h in range(HEADS):
                    wq_sb = p2.tile([P, MC, P], bf16, tag="wk")
                    nc.gpsimd.dma_start(out=wq_sb[:], in_=wq[h])
                    for q in range(QH):
                        ps = psA.tile([P, FD], fp32, tag="pp", bufs=4)
                        for m in range(MC):
                            nc.tensor.matmul(
                                ps[:], wq_sb[:, m, :],
                                xT[m][:, q * FD:(q + 1) * FD],
                                start=(m == 0), stop=(m == MC - 1),
                            )
                        nc.vector.tensor_copy(
                            qt_sb[h][:, q * FD:(q + 1) * FD], ps[:]
                        )

        # ---------------- Phase 4: attention -------------------------------
        with tc.tile_pool(name="proj", bufs=2) as pr:
            wo_sb = pr.tile([P, MC, DIM], bf16, tag="wo", bufs=1)
            nc.sync.dma_start(
                out=wo_sb[:],
                in_=wo.ap().rearrange("(hh p) d -> p hh d", p=P),
            )
            with (
                tc.tile_pool(name="attn", bufs=2) as ap_,
                tc.tile_pool(name="psB", bufs=2, space="PSUM") as psB,
            ):
                for h in range(HEADS):
                    kt_sb = ap_.tile([P, N], bf16, tag="kt", bufs=3)
                    for b in range(2):
                        nc.gpsimd.dma_start(
                            out=kt_sb[:, b * LOCAL:(b + 1) * LOCAL],
                            in_=kg_h[h // HH][b][h % HH],
                        )
                    v_sb = ap_.tile([P, KCHUNKS, P], bf16, tag="vt", bufs=3)
                    hc = (h % 8) * DHEAD
                    for b in range(2):
                        nc.gpsimd.dma_start(
                            out=v_sb[:, b * RT:(b + 1) * RT, :],
                            in_=vg_h[h // 8][b][:, :, hc:hc + DHEAD].rearrange(
                                "r p d -> p r d"
                            ),
                        )
                    for q in range(QH):
                        av = psB.tile([P, FD], fp32, tag="av", bufs=2, name="av")
                        rs = psB.tile([P, FD], fp32, tag="rs", bufs=2, name="rs")
                        for kp in range(KCHUNKS // 2):
                            sim = psB.tile([P, 2, FD], fp32, tag="sim", bufs=2)
                            for j in range(2):
                                kc = kp * 2 + j
                                nc.tensor.matmul(
                                    sim[:, j, :], kt_sb[:, kc * P:(kc + 1) * P],
                                    qt_sb[h][:, q * FD:(q + 1) * FD],
                                    start=True, stop=True,
                                )
                            pT = ap_.tile([P, 2, FD], bf16, tag="pT", bufs=4)
                            nc.scalar.activation(
                                pT[:], sim[:], mybir.ActivationFunctionType.Exp
                            )
                            for j in range(2):
                                kc = kp * 2 + j
                                nc.tensor.matmul(
                                    av[:], v_sb[:, kc, :], pT[:, j, :],
                                    start=(kc == 0), stop=(kc == KCHUNKS - 1),
                                )
                            for j in range(2):
                                kc = kp * 2 + j
                                nc.tensor.matmul(
                                    rs[:], ones_sb[:], pT[:, j, :],
                                    start=(kc == 0), stop=(kc == KCHUNKS - 1),
                                )
                        # rs holds the key-sum broadcast across all 128
                        # partitions (ones stationary); invert at full width
                        rcb = ap_.tile([P, FD], fp32, tag="rcb")
                        nc.vector.reciprocal_approx_fast(rcb[:], rs[:])
                        nc.vector.tensor_mul(
                            avt_sb[h][:, q * FD:(q + 1) * FD], av[:], rcb[:]
                        )

            # ---------------- Phase 5: output projection -------------------
            with tc.tile_pool(name="psC", bufs=2, space="PSUM") as psC:
                for qt in range(RT):
                    ps = psC.tile([P, OC, FD], fp32, tag="po")
                    for h in range(HEADS):
                        for oc in range(OC):
                            nc.tensor.matmul(
                                ps[:, oc, :], avt_sb[h][:, qt * P:(qt + 1) * P],
                                wo_sb[:, h, oc * FD:(oc + 1) * FD],
                                start=(h == 0), stop=(h == HEADS - 1),
                            )
                    for oc in range(OC):
                        ostg = pr.tile([P, FD], fp32, tag="ostg")
                        nc.vector.tensor_copy(ostg[:], ps[:, oc, :])
                        (nc.sync if oc % 2 == 0 else nc.scalar).dma_start(
                            out=out[qt * P:(qt + 1) * P, oc * FD:(oc + 1) * FD],
                            in_=ostg[:],
                        )

    nc.compile()
    return nc


def _get_nc():
    global _CACHED_NC
    if _CACHED_NC is None:
        _CACHED_NC = build()
    return _CACHED_NC


def _make_in_maps(tokens, norm_weight, w_q, w_kv, w_out):
    tokens = np.asarray(tokens, dtype=np.float32)
    norm_weight = np.asarray(norm_weight, dtype=np.float32)
    w_q = np.asarray(w_q, dtype=np.float32)
    w_kv = np.asarray(w_kv, dtype=np.float32)
    w_out = np.asarray(w_out, dtype=np.float32)

    wq_eff = (w_q * norm_weight[:, None]) * (DHEAD ** -0.5)
    wk_eff = w_kv[:, :DIM] * norm_weight[:, None]
    wv_eff = w_kv[:, DIM:] * norm_weight[:, None]

    def pack_T(w):  # [DIM, DIM] -> [h, p, mc, d]
        t = w.reshape(MC, P, HEADS, DHEAD)
        return np.ascontiguousarray(t.transpose(2, 1, 0, 3)).astype(BF16)

    wq_p = pack_T(wq_eff)
    wk_p = pack_T(wk_eff)
    wv_b = wv_eff.astype(BF16)
    wo_b = w_out.astype(BF16)

    in_maps = []
    for c in range(NCORES):
        bi, hi = c // 2, c % 2
        tk = np.ascontiguousarray(tokens[bi, hi * LOCAL:(hi + 1) * LOCAL])
        in_maps.append(
            {"tokens": tk, "wq": wq_p, "wk": wk_p, "wv": wv_b, "wo": wo_b}
        )
    return in_maps


def _assemble(results):
    out = np.empty((B, N, DIM), np.float32)
    for c in range(NCORES):
        bi, hi = c // 2, c % 2
        out[bi, hi * LOCAL:(hi + 1) * LOCAL] = results[c]["out"]
    return out


def run(trace=False, tmpdir=None, **inputs):
    from concourse.bass_utils import run_bass_kernel_spmd

    nc = _get_nc()
    in_maps = _make_in_maps(**inputs)
    res = run_bass_kernel_spmd(
        nc, in_maps, core_ids=list(range(NCORES)), trace=trace, tmpdir=tmpdir
    )
    return _assemble(res.results), res


def kernel(**inputs):
    out, _ = run(trace=False, **inputs)
    return out



# revision 6
# speedup vs baseline: 1.0557x; 1.0557x over previous
"""Distributed Trainium2 Bass kernel: RMSNorm + 16-head attention + out-proj.

Problem (hardcoded): tokens [4, 2048, 2048] f32, DIM=2048, HEADS=16, DHEAD=128.
Sharding: 8 cores = 4 batches x 2 sequence halves. Each core owns 1024 query
rows of one batch; K/V for the full 2048 keys arrive via pair-wise AllGathers
(cores 2i and 2i+1 hold the two halves of batch i). All matmuls run in bf16
with fp32 PSUM accumulation; the rmsnorm statistics are computed in fp32 and
folded into a PE transpose (x.T @ diag(rstd)).

Softmax denominators are computed off the Tensor engine: exp chunks land in
one [P, 16, FD] SBUF tile per (head, q-half); a strided DVE tensor_reduce
sums the 16 key-chunks, and a single ones-matmul broadcasts the partition sum
across partitions (1/16th the PE cost of the ones-matmul-per-chunk scheme).
The finalize chain (reduce/cast/bcast/recip/mul) is software-pipelined one
unit behind the sim/exp/av stream so the PE never waits on it; AV matmuls lag
their exp by 2 key-chunks so the scalar engine's exp latency is hidden.

Host-side folding: norm_weight and the q-scale (dhead^-0.5) are folded into
the projection weight matrices.

Queue discipline: weight DMAs go on nc.sync / nc.scalar (HWDGE) and complete
before the collectives; the gather-dependent attention loads go on nc.gpsimd
(SWDGE rings) so a wait on the collective semaphore never blocks anything
through shared HWDGE ring flow-control.
"""

import os
import sys

for p in ("/opt/trn_rl_repo", "/root/.axon_site/_ro/trn_rl_repo"):
    if os.path.isdir(p) and p not in sys.path:
        sys.path.append(p)

import numpy as np
import ml_dtypes

BF16 = ml_dtypes.bfloat16

DIM = 2048
HEADS = 16
DHEAD = 128
B = 4
N = 2048
NCORES = 8
LOCAL = N // 2          # 1024 rows per core
P = 128                 # partitions
MC = DIM // P           # 16 model chunks
RT = LOCAL // P         # 8 row tiles
QH = 2                  # qrow halves of 512
FD = 512                # moving free dim
KCHUNKS = N // P        # 16 key chunks (over both blocks)
OC = DIM // FD          # 4 out-col chunks
HH = HEADS // 2         # 8 heads per gather half
EPS = 1.1920929e-07

_CACHED_NC = None


def build():
    from concourse import bacc, tile, mybir
    from concourse.masks import make_identity

    fp32 = mybir.dt.float32
    bf16 = mybir.dt.bfloat16

    nc = bacc.Bacc(
        "TRN2", target_bir_lowering=False, debug=False, num_devices=NCORES
    )

    toks = nc.dram_tensor("tokens", [LOCAL, DIM], fp32, kind="ExternalInput")
    wq = nc.dram_tensor("wq", [HEADS, P, MC, P], bf16, kind="ExternalInput")
    wk = nc.dram_tensor("wk", [HEADS, P, MC, P], bf16, kind="ExternalInput")
    wv = nc.dram_tensor("wv", [DIM, DIM], bf16, kind="ExternalInput")
    wo = nc.dram_tensor("wo", [DIM, DIM], bf16, kind="ExternalInput")
    out = nc.dram_tensor("out", [LOCAL, DIM], fp32, kind="ExternalOutput")

    # internal DRAM scratch; k split in two head-halves so each half can
    # gather as soon as its projection finishes
    kl_h = [nc.dram_tensor(f"kl{g}", [HH, P, LOCAL], bf16) for g in range(2)]
    vl_h = [nc.dram_tensor(f"vl{g}", [RT, P, DIM // 2], bf16)
            for g in range(2)]                                # local v, h-group
    kg_h = [nc.dram_tensor(f"kg{g}", [2, HH, P, LOCAL], bf16) for g in range(2)]
    vg_h = [nc.dram_tensor(f"vg{g}", [2, RT, P, DIM // 2], bf16)
            for g in range(2)]

    RG = [[0, 1], [2, 3], [4, 5], [6, 7]]

    with tile.TileContext(nc) as tc:
      with tc.tile_pool(name="persist", bufs=1) as persist:
        qt_sb = [persist.tile([P, LOCAL], bf16, tag=f"qt{h}", name=f"qt{h}")
                 for h in range(HEADS)]
        avt_sb = [persist.tile([P, LOCAL], bf16, tag=f"avt{h}", name=f"avt{h}")
                  for h in range(HEADS)]
        ones_sb = persist.tile([P, P], bf16, tag="ones")
        nc.vector.memset(ones_sb[:], 1.0)
        eps_sb = persist.tile([P, 1], fp32, tag="eps")
        nc.vector.memset(eps_sb[:], EPS)

        # ------- Phase 1: rmsnorm fused into a PE transpose (x.T @ diag) ----
        with tc.tile_pool(name="xt", bufs=1) as xtp:
            xT = xtp.tile([P, MC, LOCAL], bf16, tag="xt", name="xT")
            with (
                tc.tile_pool(name="p1", bufs=2) as p1,
                tc.tile_pool(name="stat", bufs=4) as stat,
                tc.tile_pool(name="psT", bufs=2, space="PSUM") as psT,
            ):
                ident = p1.tile([P, P], bf16, tag="ident", bufs=1)
                make_identity(nc, ident[:])
                for rt in range(RT):
                    x = p1.tile([P, DIM], fp32, tag="x", bufs=3)
                    (nc.sync if rt % 2 == 0 else nc.scalar).dma_start(
                        out=x[:], in_=toks[rt * P:(rt + 1) * P, :]
                    )
                    sq = p1.tile([P, DIM], fp32, tag="sq")
                    ssq = stat.tile([P, 1], fp32, tag="ssq")
                    nc.scalar.activation(
                        sq[:], x[:], mybir.ActivationFunctionType.Square,
                        accum_out=ssq[:],
                    )
                    std = stat.tile([P, 1], fp32, tag="std")
                    nc.scalar.activation(
                        std[:], ssq[:], mybir.ActivationFunctionType.Sqrt,
                        bias=eps_sb[:], scale=1.0 / DIM,
                    )
                    rstd = stat.tile([P, 1], fp32, tag="rstd")
                    nc.vector.reciprocal(rstd[:], std[:])
                    diag = stat.tile([P, P], bf16, tag="diag")
                    nc.vector.tensor_scalar_mul(diag[:], ident[:], rstd[:])
                    xb = p1.tile([P, DIM], bf16, tag="xb")
                    nc.vector.tensor_copy(xb[:], x[:])
                    # xT[:, m, rows] = (x[:, m-chunk]).T @ diag(rstd)
                    for g in range(MC // 4):
                        pt = psT.tile([P, 4, P], fp32, tag="pt")
                        for j in range(4):
                            m = 4 * g + j
                            nc.tensor.matmul(
                                pt[:, j, :], xb[:, m * P:(m + 1) * P], diag[:],
                                start=True, stop=True,
                            )
                        dst = xT[:, 4 * g:4 * (g + 1), rt * P:(rt + 1) * P]
                        nc.vector.tensor_copy(dst, pt[:])
                # pre-warm the scalar engine's Exp table so attention's first
                # exp doesn't pay the ACT_TABLE_LOAD on the critical path
                dume = stat.tile([P, 1], fp32, tag="dume")
                nc.scalar.activation(
                    dume[:], eps_sb[:], mybir.ActivationFunctionType.Exp
                )

            # ---------------- Phase 2: k^T and v projections ---------------
            with (
                tc.tile_pool(name="p2", bufs=2) as p2,
                tc.tile_pool(name="psA", bufs=3, space="PSUM") as psA,
            ):
                wv_full = p2.tile([P, MC, DIM], bf16, tag="wvf", bufs=1)
                nc.scalar.dma_start(
                    out=wv_full[:],
                    in_=wv.ap().rearrange("(mc p) d -> p mc d", p=P),
                )
                # k^T: stationary = wk chunks, moving = xT; each head-half
                # gathers as soon as it is projected
                for g in range(2):
                    for hh in range(HH):
                        h = g * HH + hh
                        wk_sb = p2.tile([P, MC, P], bf16, tag="wk")
                        nc.sync.dma_start(out=wk_sb[:], in_=wk[h])
                        for q in range(QH):
                            ps = psA.tile([P, FD], fp32, tag="pp", bufs=4)
                            for m in range(MC):
                                nc.tensor.matmul(
                                    ps[:], wk_sb[:, m, :],
                                    xT[:, m, q * FD:(q + 1) * FD],
                                    start=(m == 0), stop=(m == MC - 1),
                                )
                            stg = p2.tile([P, FD], bf16, tag="stg", bufs=8)
                            nc.vector.tensor_copy(stg[:], ps[:])
                            nc.sync.dma_start(
                                out=kl_h[g].ap()[hh][:, q * FD:(q + 1) * FD],
                                in_=stg[:],
                            )
                    nc.gpsimd.collective_compute(
                        "AllGather", mybir.AluOpType.bypass,
                        replica_groups=RG,
                        ins=[kl_h[g].ap().opt()],
                        outs=[kg_h[g].ap().opt()],
                    )
                # v: stationary = xT chunks, moving = wv_full[mc, oc] slices
                # gathered per head-group so attention h<8 starts sooner
                for g in range(2):
                    for oc in (2 * g, 2 * g + 1):
                        for rt in range(RT):
                            ps = psA.tile([P, FD], fp32, tag="pp", bufs=4)
                            for m in range(MC):
                                nc.tensor.matmul(
                                    ps[:], xT[:, m, rt * P:(rt + 1) * P],
                                    wv_full[:, m, oc * FD:(oc + 1) * FD],
                                    start=(m == 0), stop=(m == MC - 1),
                                )
                            stg = p2.tile([P, FD], bf16, tag="stg", bufs=8)
                            nc.vector.tensor_copy(stg[:], ps[:])
                            nc.sync.dma_start(
                                out=vl_h[g].ap()[rt][
                                    :, (oc % 2) * FD:(oc % 2 + 1) * FD
                                ],
                                in_=stg[:],
                            )
                    nc.gpsimd.collective_compute(
                        "AllGather", mybir.AluOpType.bypass,
                        replica_groups=RG,
                        ins=[vl_h[g].ap().opt()],
                        outs=[vg_h[g].ap().opt()],
                    )

                # ---------------- Phase 3: q^T projection ------------------
                for h in range(HEADS):
                    wq_sb = p2.tile([P, MC, P], bf16, tag="wk")
                    nc.gpsimd.dma_start(out=wq_sb[:], in_=wq[h])
                    for q in range(QH):
                        ps = psA.tile([P, FD], fp32, tag="pp", bufs=4)
                        for m in range(MC):
                            nc.tensor.matmul(
                                ps[:], wq_sb[:, m, :],
                                xT[:, m, q * FD:(q + 1) * FD],
                                start=(m == 0), stop=(m == MC - 1),
                            )
                        nc.vector.tensor_copy(
                            qt_sb[h][:, q * FD:(q + 1) * FD], ps[:]
                        )

        # ---------------- Phase 4: attention -------------------------------
        with tc.tile_pool(name="proj", bufs=2) as pr:
            wo_sb = pr.tile([P, MC, DIM], bf16, tag="wo", bufs=1)
            nc.sync.dma_start(
                out=wo_sb[:],
                in_=wo.ap().rearrange("(hh p) d -> p hh d", p=P),
            )
            with (
                tc.tile_pool(name="attn", bufs=2) as ap_,
                tc.tile_pool(name="psB", bufs=2, space="PSUM") as psB,
            ):
                # finalize chain of the previous (h, q) unit: chunk-sum on
                # DVE, partition-sum broadcast via a single ones-matmul,
                # reciprocal, and the avt normalize. Emitted one unit late so
                # the PE's bcast matmul never waits on the DVE chain.
                def make_fin(h, q, av, pT):
                    def fin():
                        # contiguous bf16 add tree: 16 chunks -> 1
                        t8 = ap_.tile([P, 8, FD], bf16, tag="t8", bufs=1,
                                      name="t8")
                        nc.vector.tensor_add(
                            t8[:], pT[:, 0:8, :], pT[:, 8:16, :]
                        )
                        t4 = ap_.tile([P, 4, FD], bf16, tag="t4", bufs=1,
                                      name="t4")
                        nc.vector.tensor_add(
                            t4[:], t8[:, 0:4, :], t8[:, 4:8, :]
                        )
                        t2 = ap_.tile([P, 2, FD], bf16, tag="t2", bufs=1,
                                      name="t2")
                        nc.vector.tensor_add(
                            t2[:], t4[:, 0:2, :], t4[:, 2:4, :]
                        )
                        s1b = ap_.tile([P, FD], bf16, tag="s1b", bufs=2,
                                       name="s1b")
                        nc.vector.tensor_add(s1b[:], t2[:, 0, :], t2[:, 1, :])
                        rsp = psB.tile([P, 2, FD], fp32, tag="sim", bufs=3,
                                       name="rsp")
                        nc.tensor.matmul(
                            rsp[:, 0, :], ones_sb[:], s1b[:],
                            start=True, stop=True,
                        )
                        rcb = ap_.tile([P, FD], fp32, tag="rcb", bufs=2,
                                       name="rcb")
                        nc.vector.reciprocal_approx_fast(rcb[:], rsp[:, 0, :])
                        nc.vector.tensor_mul(
                            avt_sb[h][:, q * FD:(q + 1) * FD], av[:], rcb[:]
                        )
                    return fin

                fin_prev = None
                for h in range(HEADS):
                    kt_sb = ap_.tile([P, N], bf16, tag="kt", bufs=2)
                    for b in range(2):
                        nc.gpsimd.dma_start(
                            out=kt_sb[:, b * LOCAL:(b + 1) * LOCAL],
                            in_=kg_h[h // HH][b][h % HH],
                        )
                    v_sb = ap_.tile([P, KCHUNKS, P], bf16, tag="vt", bufs=2)
                    hc = (h % 8) * DHEAD
                    for b in range(2):
                        nc.gpsimd.dma_start(
                            out=v_sb[:, b * RT:(b + 1) * RT, :],
                            in_=vg_h[h // 8][b][:, :, hc:hc + DHEAD].rearrange(
                                "r p d -> p r d"
                            ),
                        )
                    for q in range(QH):
                        av = psB.tile([P, FD], fp32, tag="av", bufs=2,
                                      name="av")
                        pT = ap_.tile([P, KCHUNKS, FD], bf16, tag="pT",
                                      bufs=2, name="pT")

                        def av_mm(kc):
                            nc.tensor.matmul(
                                av[:], v_sb[:, kc, :], pT[:, kc, :],
                                start=(kc == 0), stop=(kc == KCHUNKS - 1),
                            )

                        for kp in range(KCHUNKS // 2):
                            sim = psB.tile([P, 2, FD], fp32, tag="sim",
                                           bufs=3)
                            for j in range(2):
                                kc = kp * 2 + j
                                nc.tensor.matmul(
                                    sim[:, j, :], kt_sb[:, kc * P:(kc + 1) * P],
                                    qt_sb[h][:, q * FD:(q + 1) * FD],
                                    start=True, stop=True,
                                )
                            nc.scalar.activation(
                                pT[:, 2 * kp:2 * kp + 2, :], sim[:],
                                mybir.ActivationFunctionType.Exp,
                            )
                            # finalize the previous unit early in this one
                            if kp == 1 and fin_prev is not None:
                                fin_prev()
                                fin_prev = None
                            # AV lags exp by 2 key-chunks so the scalar
                            # engine stays ahead of the PE
                            if kp >= 2:
                                av_mm(2 * kp - 4)
                                av_mm(2 * kp - 3)
                        for kc in (KCHUNKS - 4, KCHUNKS - 3,
                                   KCHUNKS - 2, KCHUNKS - 1):
                            av_mm(kc)
                        fin_prev = make_fin(h, q, av, pT)
                fin_prev()
                fin_prev = None

            # ---------------- Phase 5: output projection -------------------
            with tc.tile_pool(name="psC", bufs=2, space="PSUM") as psC:
                for qt in range(RT):
                    ps = psC.tile([P, OC, FD], fp32, tag="po")
                    for h in range(HEADS):
                        for oc in range(OC):
                            nc.tensor.matmul(
                                ps[:, oc, :], avt_sb[h][:, qt * P:(qt + 1) * P],
                                wo_sb[:, h, oc * FD:(oc + 1) * FD],
                                start=(h == 0), stop=(h == HEADS - 1),
                            )
                    for oc in range(OC):
                        ostg = pr.tile([P, FD], fp32, tag="ostg")
                        nc.vector.tensor_copy(ostg[:], ps[:, oc, :])
                        (nc.sync if oc % 2 == 0 else nc.scalar).dma_start(
                            out=out[qt * P:(qt + 1) * P, oc * FD:(oc + 1) * FD],
                            in_=ostg[:],
                        )

    nc.compile()
    return nc


def _get_nc():
    global _CACHED_NC
    if _CACHED_NC is None:
        _CACHED_NC = build()
    return _CACHED_NC


def _make_in_maps(tokens, norm_weight, w_q, w_kv, w_out):
    tokens = np.asarray(tokens, dtype=np.float32)
    norm_weight = np.asarray(norm_weight, dtype=np.float32)
    w_q = np.asarray(w_q, dtype=np.float32)
    w_kv = np.asarray(w_kv, dtype=np.float32)
    w_out = np.asarray(w_out, dtype=np.float32)

    wq_eff = (w_q * norm_weight[:, None]) * (DHEAD ** -0.5)
    wk_eff = w_kv[:, :DIM] * norm_weight[:, None]
    wv_eff = w_kv[:, DIM:] * norm_weight[:, None]

    def pack_T(w):  # [DIM, DIM] -> [h, p, mc, d]
        t = w.reshape(MC, P, HEADS, DHEAD)
        return np.ascontiguousarray(t.transpose(2, 1, 0, 3)).astype(BF16)

    wq_p = pack_T(wq_eff)
    wk_p = pack_T(wk_eff)
    wv_b = wv_eff.astype(BF16)
    wo_b = w_out.astype(BF16)

    in_maps = []
    for c in range(NCORES):
        bi, hi = c // 2, c % 2
        tk = np.ascontiguousarray(tokens[bi, hi * LOCAL:(hi + 1) * LOCAL])
        in_maps.append(
            {"tokens": tk, "wq": wq_p, "wk": wk_p, "wv": wv_b, "wo": wo_b}
        )
    return in_maps


def _assemble(results):
    out = np.empty((B, N, DIM), np.float32)
    for c in range(NCORES):
        bi, hi = c // 2, c % 2
        out[bi, hi * LOCAL:(hi + 1) * LOCAL] = results[c]["out"]
    return out


def run(trace=False, tmpdir=None, **inputs):
    from concourse.bass_utils import run_bass_kernel_spmd

    nc = _get_nc()
    in_maps = _make_in_maps(**inputs)
    res = run_bass_kernel_spmd(
        nc, in_maps, core_ids=list(range(NCORES)), trace=trace, tmpdir=tmpdir
    )
    return _assemble(res.results), res


def kernel(**inputs):
    out, _ = run(trace=False, **inputs)
    return out
